# revision 1
# baseline (speedup 1.0000x reference)
"""Trainium2 Bass kernel for nn_KeySelect (sparse_attention).

Sharding: 8 shards = (4 batches) x (2 spatial H-halves). Each core gets a
48-row padded slab (40 compute rows + 4-row halo each side, zero-filled
outside the image) and computes the full conv chain for its half with no
collectives; halo rows that would be wrong at the interior cut are computed
but discarded on the host (validity windows shrink by 1 per 3x3 conv and 4
for the 9x9 local-attention weighting).

Layout: every activation buffer is [C<=128 partitions, 48*72] fp32 — H rows
of 72 (64 valid cols + 4 zero pad cols each side). A 3x3 conv is 9
PSUM-accumulated matmuls at free-dim offsets dh*72+dw; the 9x9 weighting
uses offsets in the same coordinate system.

Weighting out[c,p] = sum_k A[p,k] * x[c, p+d(k)] runs on DVE: attention rows
are broadcast across partitions by a K=2 PE outer-product (selector matrix),
pairing taps k and k+45 (constant offset 360 = 5*72) so all 128 partitions
are used: upper 64 partitions accumulate tap k, lower 64 tap k+45 against a
row-shifted duplicate copy of x.

Output per core: per-row spatial sums of relu(bn5(conv5)) [2,128,40]; host
selects each core's 32 valid rows, finishes the mean and the two tiny FCs.
"""

import numpy as np

import concourse.bacc as bacc
import concourse.bass as bass
import concourse.mybir as mybir
from concourse import tile
from concourse.alu_op_type import AluOpType
from concourse.bass_utils import run_bass_kernel_spmd

F32 = mybir.dt.float32
PADW, PADH = 72, 48
PADN = PADW * PADH  # 3456
DUPB = 360          # lower-half row shift: delta(k+45)-delta(k) = 5*72
DUPN = 4096         # x2dup free size (slack for shifted reads)
ACC0 = 4 * PADW     # 288: first out element (row 4, col 0)
ACCN = 40 * PADW    # 2880: 40 output rows
WBLK = 480          # weighting block (6 blocks of 480 = 2880)
BLOCKS = [(4, 7), (11, 7), (18, 7), (25, 7), (32, 7), (39, 5)]
SHIFTS = [(dh, dw) for dh in (-1, 0, 1) for dw in (-1, 0, 1)]
NPAIR = 45          # weighting groups: (k, k+45) for k<36, singles 36..44
BN_EPS = 1e-5
H = W = 64
ROWS = 40           # compute rows per core
HALO = 4


def _delta(k):
    return (k // 9 - 4) * PADW + (k % 9 - 4)


# ---------------------------------------------------------------- program --

def _build_program():
    nc = bacc.Bacc("TRN2", target_bir_lowering=False, debug=False)

    lk = nc.dram_tensor("lk", [8, 128, PADN], F32, kind="ExternalInput")[:]
    ln = nc.dram_tensor("ln", [8, 128, PADN], F32, kind="ExternalInput")[:]
    asb_d = nc.dram_tensor("asb", [NPAIR, 2, PADN], F32, kind="ExternalInput")[:]
    w1 = nc.dram_tensor("w1", [8, 128, 2304], F32, kind="ExternalInput")[:]
    w2 = nc.dram_tensor("w2", [128, 1152], F32, kind="ExternalInput")[:]
    w3 = nc.dram_tensor("w3", [64, 2304], F32, kind="ExternalInput")[:]
    w4 = nc.dram_tensor("w4", [8, 128, 2304], F32, kind="ExternalInput")[:]
    w5 = nc.dram_tensor("w5", [2, 128, 2304], F32, kind="ExternalInput")[:]
    sel_d = nc.dram_tensor("sel", [2, 128], F32, kind="ExternalInput")[:]
    fold_d = nc.dram_tensor("fold", [128, 64], F32, kind="ExternalInput")[:]
    bnp_d = nc.dram_tensor("bnp", [128, 18], F32, kind="ExternalInput")[:]
    osum = nc.dram_tensor("osum", [2, 128, 40], F32, kind="ExternalOutput")[:]

    # bnp columns: [c1sc0,c1sc1,c1sh0,c1sh1, c2sc,c2sh, c3sc0,c3sc1,c3sh0,
    #               c3sh1, c4sc0,c4sc1,c4sh0,c4sh1, c5sc0,c5sc1,c5sh0,c5sh1]
    C1SC, C1SH, C2SC, C2SH = 0, 2, 4, 5
    C3SC, C3SH, C4SC, C4SH, C5SC, C5SH = 6, 8, 10, 12, 14, 16

    with tile.TileContext(nc) as tc:
        with (
            tc.tile_pool(name="sb", bufs=1) as sb,
            tc.tile_pool(name="cps", bufs=6, space="PSUM") as cps,
            tc.tile_pool(name="aps", bufs=2, space="PSUM") as aps,
            tc.tile_pool(name="tmp", bufs=4) as tmpp,
            tc.tile_pool(name="scr", bufs=4) as scrp,
        ):
            slabs = [sb.tile([128, PADN], F32, name=f"slab{i}", tag=f"slab{i}") for i in range(2)]
            wst = [sb.tile([128, 2304], F32, name=f"wst{i}", tag=f"wst{i}") for i in range(2)]
            c1 = [sb.tile([128, PADN], F32, name=f"c1_{i}", tag=f"c1_{i}") for i in range(2)]
            ybuf = [sb.tile([128, PADN], F32, name=f"y{i}", tag=f"y{i}") for i in range(2)]
            x2dup = sb.tile([128, DUPN], F32, name="x2dup", tag="x2dup")
            accd = sb.tile([128, PADN], F32, name="accd", tag="accd")
            w2sb = sb.tile([128, 1152], F32, name="w2sb", tag="w2sb")
            w3sb = sb.tile([64, 2304], F32, name="w3sb", tag="w3sb")
            sel = sb.tile([2, 128], F32, name="sel", tag="sel")
            fold = sb.tile([128, 64], F32, name="fold", tag="fold")
            bnp = sb.tile([128, 18], F32, name="bnp", tag="bnp")
            sums = sb.tile([128, 80], F32, name="sums", tag="sums")

            # constants + zero-init of pad regions
            nc.gpsimd.dma_start(w2sb[:], w2)
            nc.gpsimd.dma_start(w3sb[:], w3)
            nc.gpsimd.dma_start(sel[:], sel_d)
            nc.gpsimd.dma_start(fold[:], fold_d)
            nc.gpsimd.dma_start(bnp[:], bnp_d)
            for t in (c1[0], c1[1], ybuf[0], ybuf[1], accd):
                nc.gpsimd.memset(t[:], 0.0)
            nc.gpsimd.memset(x2dup[:], 0.0)

            def r3(ap):  # [P, n*72] -> [P, n, 72]
                return ap.rearrange("p (r c) -> p r c", c=PADW)

            def conv_mms(rhs_of, lhsT_of, nkt, psts, first, last):
                """nkt ktiles x 6 blocks x 9 shifts accumulated into psts."""
                for ti in range(nkt):
                    rhs = rhs_of(ti)
                    for bi, (r0, nr) in enumerate(BLOCKS):
                        for si, (dh, dw) in enumerate(SHIFTS):
                            o = (r0 + dh) * PADW + dw
                            nc.tensor.matmul(
                                out=psts[bi][:, : nr * PADW],
                                lhsT=lhsT_of(ti, si),
                                rhs=rhs[:, o : o + nr * PADW],
                                start=(first and ti == 0 and si == 0),
                                stop=(last and ti == nkt - 1 and si == 8),
                            )
                        yield ti, bi

            def evict(psts, dst, m, sc_col, sh_col, base=0):
                """PSUM -> dst with fused BN+ReLU, valid cols only."""
                for bi, (r0, nr) in enumerate(BLOCKS):
                    nc.scalar.activation(
                        out=r3(dst[:m, base : base + PADN])[:, r0 : r0 + nr, 4:68],
                        in_=r3(psts[bi][:m, : nr * PADW])[:, :, 4:68],
                        func=mybir.ActivationFunctionType.Relu,
                        scale=bnp[:m, sc_col : sc_col + 1],
                        bias=bnp[:m, sh_col : sh_col + 1],
                    )

            # ---- conv1: 1024 -> 256, rhs = lk slabs (streamed, 2 passes) --
            for co in range(2):
                psts = [cps.tile([128, nr * PADW], F32, name="cp", tag="cp") for _, nr in BLOCKS]

                def lk_rhs(ti, _co=co):
                    s = slabs[ti % 2]
                    nc.gpsimd.dma_start(s[:], lk[ti])
                    wt = wst[ti % 2]
                    nc.gpsimd.dma_start(wt[:], w1[ti])
                    return s[:]

                rhs_cache = {}

                def rhs_of(ti):
                    if ti not in rhs_cache:
                        rhs_cache[ti] = lk_rhs(ti)
                    return rhs_cache[ti]

                for _ in conv_mms(
                    rhs_of,
                    lambda ti, si, _co=co: wst[ti % 2][:, si * 256 + _co * 128 :
                                                       si * 256 + _co * 128 + 128],
                    8, psts, True, True,
                ):
                    pass
                evict(psts, c1[co][:], 128, C1SC + co, C1SH + co)

            # ---- conv2: 256 -> 64, rhs = c1, out -> x2dup upper @DUPB -----
            psts = [cps.tile([64, nr * PADW], F32, name="cp", tag="cp") for _, nr in BLOCKS]
            for _ in conv_mms(
                lambda ti: c1[ti][:],
                lambda ti, si: w2sb[:, ti * 576 + si * 64 : ti * 576 + si * 64 + 64],
                2, psts, True, True,
            ):
                pass
            evict(psts, x2dup[:, DUPB : DUPB + PADN], 64, C2SC, C2SH)
            # lower half copy: x2dup[64:128, q] = x2[q] (DMA: cross-partition)
            nc.gpsimd.dma_start(
                x2dup[64:128, 0:PADN], x2dup[0:64, DUPB : DUPB + PADN]
            )

            # ---- conv4 (y branch) interleaved with weighting A_rep/FMA ----
            arep_units = [(g, j) for g in range(NPAIR) for j in range(6)]
            aidx = 0
            cur_ag = [None]

            def emit_arep(n):
                nonlocal aidx
                for _ in range(n):
                    if aidx >= len(arep_units):
                        return
                    g, j = arep_units[aidx]
                    aidx += 1
                    if j == 0:
                        ag = scrp.tile([2, PADN], F32, name="asbg", tag="asbg",
                                       bufs=2)
                        nc.gpsimd.dma_start(ag[:], asb_d[g])
                        cur_ag[0] = ag
                    ag = cur_ag[0]
                    s = ACC0 + j * WBLK
                    pa = aps.tile([128, WBLK], F32, name="arep", tag="arep")
                    nc.tensor.matmul(
                        out=pa[:], lhsT=sel[:],
                        rhs=ag[0:2, s : s + WBLK],
                        start=True, stop=True,
                    )
                    x = DUPB + _delta(g) + s
                    tt = tmpp.tile([128, WBLK], F32, name="wt", tag="wt")
                    nc.vector.tensor_tensor(
                        out=tt[:], in0=x2dup[:, x : x + WBLK], in1=pa[:],
                        op=AluOpType.mult,
                    )
                    nc.vector.tensor_tensor(
                        out=accd[:, s : s + WBLK], in0=accd[:, s : s + WBLK],
                        in1=tt[:], op=AluOpType.add,
                    )

            for co in range(2):
                psts = [cps.tile([128, nr * PADW], F32, name="cp", tag="cp") for _, nr in BLOCKS]
                rhs_cache = {}

                def rhs_of(ti):
                    if ti not in rhs_cache:
                        s = slabs[ti % 2]
                        nc.gpsimd.dma_start(s[:], ln[ti])
                        wt = wst[ti % 2]
                        nc.gpsimd.dma_start(wt[:], w4[ti])
                        rhs_cache[ti] = s[:]
                    return rhs_cache[ti]

                for _ti, _bi in conv_mms(
                    rhs_of,
                    lambda ti, si, _co=co: wst[ti % 2][:, si * 256 + _co * 128 :
                                                       si * 256 + _co * 128 + 128],
                    8, psts, True, True,
                ):
                    emit_arep(3)
                evict(psts, ybuf[co][:], 128, C4SC + co, C4SH + co)
            emit_arep(len(arep_units))

            # collapse pair halves: wout = acc_upper + acc_lower (PE fold)
            for o in range(0, PADN, 512):
                n = min(512, PADN - o)
                pc = cps.tile([64, 512], F32, name="cp", tag="cp")
                nc.tensor.matmul(
                    out=pc[:, :n], lhsT=fold[:], rhs=accd[:, o : o + n],
                    start=True, stop=True,
                )
                nc.scalar.activation(
                    out=accd[0:64, o : o + n], in_=pc[:, :n],
                    func=mybir.ActivationFunctionType.Copy,
                )

            # ---- conv3: 64 -> 256, rhs = wout (accd upper), out -> c1 -----
            for co in range(2):
                psts = [cps.tile([128, nr * PADW], F32, name="cp", tag="cp") for _, nr in BLOCKS]
                for _ in conv_mms(
                    lambda ti: accd[0:64, :],
                    lambda ti, si, _co=co: w3sb[:, si * 256 + _co * 128 :
                                                si * 256 + _co * 128 + 128],
                    1, psts, True, True,
                ):
                    pass
                evict(psts, c1[co][:], 128, C3SC + co, C3SH + co)

            # ---- d = x3 - y (in place, valid rows incl. pad cols) ---------
            for co in range(2):
                nc.vector.tensor_tensor(
                    out=c1[co][:, ACC0 : ACC0 + ACCN],
                    in0=c1[co][:, ACC0 : ACC0 + ACCN],
                    in1=ybuf[co][:, ACC0 : ACC0 + ACCN],
                    op=AluOpType.subtract,
                )

            # ---- conv5: 256 -> 256 + BN+ReLU + per-row sums ---------------
            nc.gpsimd.dma_start(wst[0][:], w5[0])
            nc.gpsimd.dma_start(wst[1][:], w5[1])
            for co in range(2):
                psts = [cps.tile([128, nr * PADW], F32, name="cp", tag="cp") for _, nr in BLOCKS]
                for _ in conv_mms(
                    lambda ti: c1[ti][:],
                    lambda ti, si, _co=co: wst[ti][:, si * 256 + _co * 128 :
                                                   si * 256 + _co * 128 + 128],
                    2, psts, True, True,
                ):
                    pass
                for bi, (r0, nr) in enumerate(BLOCKS):
                    for rr in range(nr):
                        ridx = (r0 - 4) + rr
                        c5s = scrp.tile([128, 64], F32, name="c5s", tag="c5s")
                        nc.scalar.activation(
                            out=c5s[:],
                            in_=r3(psts[bi][:, : nr * PADW])[:, rr, 4:68],
                            func=mybir.ActivationFunctionType.Relu,
                            scale=bnp[:, C5SC + co : C5SC + co + 1],
                            bias=bnp[:, C5SH + co : C5SH + co + 1],
                            accum_out=sums[:, co * 40 + ridx : co * 40 + ridx + 1],
                        )
            for co in range(2):
                nc.gpsimd.dma_start(osum[co], sums[:, co * 40 : co * 40 + 40])

    nc.compile()
    return nc


# ------------------------------------------------------------- host side --

def _pad_slab(x_bchw, g0):
    """[1024, 64, 64] -> [8, 128, 48*72], rows g0-4 .. g0+44 zero-padded."""
    out = np.zeros((1024, PADH, PADW), np.float32)
    lo, hi = max(0, g0 - HALO), min(H, g0 + ROWS + HALO)
    out[:, lo - (g0 - HALO) : hi - (g0 - HALO), 4:68] = x_bchw[:, lo:hi, :]
    return out.reshape(8, 128, PADN)


def _fold_bn(bn):
    g, b, m, v = [np.asarray(x, np.float32) for x in bn]
    sc = g / np.sqrt(v + BN_EPS)
    return sc, b - m * sc


def _wt(w, nkt):
    """[co, ci, 3, 3] -> [nkt, 128, 9*co] lhsT layout (free = si*Co + co)."""
    co, ci = w.shape[:2]
    return np.ascontiguousarray(
        w.reshape(co, nkt, ci // nkt, 9).transpose(1, 2, 3, 0)
    ).reshape(nkt, ci // nkt, 9 * co).astype(np.float32)


def prep_core_inputs(inputs, core):
    b, half = core // 2, core % 2
    g0 = half * 24  # local row r = global g0 + r; valid out rows differ
    lk = _pad_slab(np.asarray(inputs["low_key"][b], np.float32), g0)
    ln = _pad_slab(np.asarray(inputs["low_nonkey"][b], np.float32), g0)

    att = np.asarray(inputs["local_atten"][b], np.float32)  # [64, 64, 81]
    asb = np.zeros((NPAIR, 2, PADN), np.float32)
    a_loc = np.zeros((81, PADH, PADW), np.float32)
    a_loc[:, 4 : 4 + ROWS, 4:68] = att[g0 : g0 + ROWS].transpose(2, 0, 1)
    a_loc = a_loc.reshape(81, PADN)
    for g in range(NPAIR):
        asb[g, 0] = a_loc[g]
        if g + NPAIR < 81:
            asb[g, 1] = a_loc[g + NPAIR]

    w2t = _wt(np.asarray(inputs["w2"]), 2)  # [2, 128, 576]
    w2p = np.concatenate([w2t[0], w2t[1]], axis=1)  # [128, 1152]

    sel = np.zeros((2, 128), np.float32)
    sel[0, :64] = 1.0
    sel[1, 64:] = 1.0
    fold = np.zeros((128, 64), np.float32)
    fold[np.arange(64), np.arange(64)] = 1.0
    fold[64 + np.arange(64), np.arange(64)] = 1.0

    bnp = np.zeros((128, 18), np.float32)
    for i, (name, cols) in enumerate(
        [("bn1", (0, 2)), ("bn2", (4, 5)), ("bn3", (6, 8)),
         ("bn4", (10, 12)), ("bn5", (14, 16))]
    ):
        sc, sh = _fold_bn(np.asarray(inputs[name]))
        nco = sc.shape[0]
        if nco == 256:
            bnp[:, cols[0]] = sc[:128]
            bnp[:, cols[0] + 1] = sc[128:]
            bnp[:, cols[1]] = sh[:128]
            bnp[:, cols[1] + 1] = sh[128:]
        else:
            bnp[:64, cols[0]] = sc
            bnp[:64, cols[1]] = sh

    return {
        "lk": lk, "ln": ln, "asb": asb,
        "w1": _wt(np.asarray(inputs["w1"]), 8),
        "w2": w2p,
        "w3": _wt(np.asarray(inputs["w3"]), 1)[0],  # [64, 2304]
        "w4": _wt(np.asarray(inputs["w4"]), 8),
        "w5": _wt(np.asarray(inputs["w5"]), 2),
        "sel": sel, "fold": fold, "bnp": bnp,
    }


def postprocess(osums, inputs):
    """osums: list of 8 arrays [2, 128, 40] -> final [4, 1]."""
    mean = np.zeros((4, 256), np.float32)
    for core in range(8):
        b, half = core // 2, core % 2
        s = osums[core].reshape(256, 40)
        rows = slice(0, 32) if half == 0 else slice(8, 40)
        mean[b] += s[:, rows].sum(axis=1)
    mean /= float(H * W)
    fw1 = np.asarray(inputs["fw1"], np.float32)
    fb1 = np.asarray(inputs["fb1"], np.float32)
    fw2 = np.asarray(inputs["fw2"], np.float32)
    fb2 = np.asarray(inputs["fb2"], np.float32)
    out = mean @ fw1.T + fb1
    out = out @ fw2.T + fb2
    return out.astype(np.float32)


_prog_cache = {}
LAST = {}


def kernel(**inputs) -> np.ndarray:
    import os, time
    if "nc" not in _prog_cache:
        _prog_cache["nc"] = _build_program()
    nc = _prog_cache["nc"]
    in_maps = [prep_core_inputs(inputs, core) for core in range(8)]
    t0 = time.time()
    res = run_bass_kernel_spmd(
        nc, in_maps, list(range(8)), trace=bool(os.environ.get("KS_TRACE"))
    )
    LAST["spmd_s"] = time.time() - t0
    LAST["res"] = res
    return postprocess([r["osum"] for r in res.results], inputs)



# revision 35
# speedup vs baseline: 1.3096x; 1.3096x over previous
"""Trainium2 Bass kernel for nn_KeySelect (sparse_attention) — bf16 version.

Sharding: 8 shards = (4 batches) x (2 spatial H-halves). Each core gets a
48-row padded slab (40 compute rows + 4-row halo each side, zero-filled
outside the image) and computes the full conv chain for its half with no
collectives; halo rows that would be wrong at the interior cut are computed
but discarded on the host (validity windows shrink by 1 per 3x3 conv and 4
for the 9x9 local-attention weighting).

Layout: every activation buffer is [C<=128 partitions, 48*72] — H rows
of 72 (64 valid cols + 4 zero pad cols each side). A 3x3 conv is 9
PSUM-accumulated matmuls at free-dim offsets dh*72+dw; the 9x9 weighting
uses offsets in the same coordinate system.

All matmul operands are bf16 (PSUM accumulation stays fp32): HW streams
bf16 at ~1 cycle/row vs ~6 for fp32. Conv matmuls stream a strided
[K, nr, 66] window (cols 3..69 of each 72-col row) so the pad columns are
never pushed through the PE. Activations evict from PSUM to bf16 via the
scalar engine's fused BN+ReLU. DMA traffic is cut 4x vs the fp32 baseline:
bf16 halves it, and keeping all 8 input slabs + weight tiles resident in
SBUF lets conv1/conv4 stream lk/ln + w1/w4 from HBM exactly once. Big DMAs
ride the two HW-DGE queues (scalar/sync engines), alternating per tile.

Weighting out[c,p] = sum_k A[p,k] * x[c, p+d(k)]: attention rows live in a
resident [90, PADN] tile; a per-group [90,128] 0/1 selector (lhsT slice of
one resident tile) broadcasts the pair (k, k+45) across 128 partitions via
the PE, pairing taps at constant offset 360 = 5*72 against a row-shifted
duplicate of x2 so all 128 lanes are used. The FMA is split across engines:
mult on DVE (PSUM operand), accumulate-add on gpsimd, both bf16.

Output per core: relu(bn5(conv5)) blocks land in SBUF bf16; one segmented
tensor_reduce per channel-half yields per-row sums [2,128,40]; host selects
each core's 32 valid rows, finishes the mean and the two tiny FCs.
"""

import numpy as np

import concourse.bacc as bacc
import concourse.mybir as mybir
from concourse import tile
from concourse.alu_op_type import AluOpType
from concourse.bass_utils import run_bass_kernel_spmd
from bass_rust import AxisListType

F32 = mybir.dt.float32
BF16 = mybir.dt.bfloat16
NPBF16 = mybir.dt.np(BF16)
PADW, PADH = 72, 48
PADN = PADW * PADH  # 3456
DUPB = 360          # lower-half row shift: delta(k+45)-delta(k) = 5*72
DUPN = 4096         # x2dup free size (slack for shifted reads)
ACC0 = 4 * PADW     # 288: first out element (row 4, col 0)
ACCN = 40 * PADW    # 2880: 40 output rows
WBLK = 480          # weighting block (6 blocks of 480 = 2880)
BLOCKS = [(4, 7), (11, 7), (18, 7), (25, 7), (32, 7), (39, 5)]
TRIM, TW = 3, 66    # conv matmuls stream cols 3..69 of each 72-col row
SHIFTS = [(dh, dw) for dh in (-1, 0, 1) for dw in (-1, 0, 1)]
NPAIR = 45          # weighting groups: (k, k+45) for k<36, singles 36..44
BN_EPS = 1e-5
H = W = 64
ROWS = 40           # compute rows per core
HALO = 4


def _delta(k):
    return (k // 9 - 4) * PADW + (k % 9 - 4)


# ---------------------------------------------------------------- program --

def _build_program():
    nc = bacc.Bacc("TRN2", target_bir_lowering=False, debug=False)

    lk = nc.dram_tensor("lk", [8, 128, PADN], BF16, kind="ExternalInput")[:]
    ln = nc.dram_tensor("ln", [8, 128, PADN], BF16, kind="ExternalInput")[:]
    asb_d = nc.dram_tensor("asb", [2 * NPAIR, PADN], BF16, kind="ExternalInput")[:]
    w1 = nc.dram_tensor("w1", [8, 128, 2304], BF16, kind="ExternalInput")[:]
    w2 = nc.dram_tensor("w2", [128, 1152], BF16, kind="ExternalInput")[:]
    w3 = nc.dram_tensor("w3", [64, 2304], BF16, kind="ExternalInput")[:]
    w4 = nc.dram_tensor("w4", [8, 128, 2304], BF16, kind="ExternalInput")[:]
    w5 = nc.dram_tensor("w5", [2, 128, 2304], BF16, kind="ExternalInput")[:]
    sel_d = nc.dram_tensor("sel", [2 * NPAIR, NPAIR * 128], BF16, kind="ExternalInput")[:]
    fold_d = nc.dram_tensor("fold", [128, 64], BF16, kind="ExternalInput")[:]
    bnp_d = nc.dram_tensor("bnp", [128, 18], F32, kind="ExternalInput")[:]
    osum = nc.dram_tensor("osum", [2, 128, 40], F32, kind="ExternalOutput")[:]

    # bnp columns: [c1sc0,c1sc1,c1sh0,c1sh1, c2sc,c2sh, c3sc0,c3sc1,c3sh0,
    #               c3sh1, c4sc0,c4sc1,c4sh0,c4sh1, c5sc0,c5sc1,c5sh0,c5sh1]
    C1SC, C1SH, C2SC, C2SH = 0, 2, 4, 5
    C3SC, C3SH, C4SC, C4SH, C5SC, C5SH = 6, 8, 10, 12, 14, 16

    with tile.TileContext(nc) as tc:
        with (
            tc.tile_pool(name="sb", bufs=1) as sb,
            tc.tile_pool(name="cps", bufs=6, space="PSUM") as cps,
            tc.tile_pool(name="aps", bufs=2, space="PSUM") as aps,
            tc.tile_pool(name="tmp", bufs=4) as tmpp,
        ):
            slabs = [sb.tile([128, PADN], BF16, name=f"slab{i}", tag=f"slab{i}") for i in range(8)]
            wst = [sb.tile([128, 2304], BF16, name=f"wst{i}", tag=f"wst{i}") for i in range(8)]
            c1 = [sb.tile([128, PADN], BF16, name=f"c1_{i}", tag=f"c1_{i}") for i in range(2)]
            ybuf = [sb.tile([128, PADN], BF16, name=f"y{i}", tag=f"y{i}") for i in range(2)]
            x2dup = sb.tile([128, DUPN], BF16, name="x2dup", tag="x2dup")
            accd = sb.tile([128, PADN], BF16, name="accd", tag="accd")
            wout = sb.tile([64, PADN], BF16, name="wout", tag="wout")
            asbsb = sb.tile([2 * NPAIR, PADN], BF16, name="asbsb", tag="asbsb")
            w2sb = sb.tile([128, 1152], BF16, name="w2sb", tag="w2sb")
            w3sb = sb.tile([64, 2304], BF16, name="w3sb", tag="w3sb")
            sel = sb.tile([2 * NPAIR, NPAIR * 128], BF16, name="sel", tag="sel")
            fold = sb.tile([128, 64], BF16, name="fold", tag="fold")
            bnp = sb.tile([128, 18], F32, name="bnp", tag="bnp")
            sums = sb.tile([128, 80], F32, name="sums", tag="sums")
            c5b = sb.tile([128, 2560], BF16, name="c5b", tag="c5b")

            # stream lk slabs + w1 tiles once on the two HW-DGE queues; they
            # stay resident for both output-channel halves. ln/w4 reuse the
            # same buffers later. Issued first so conv1's inputs lead each
            # queue; small constants follow.
            for ti in range(8):
                (nc.scalar if ti % 2 == 0 else nc.sync).dma_start(slabs[ti][:], lk[ti])
                (nc.sync if ti % 2 == 0 else nc.scalar).dma_start(wst[ti][:], w1[ti])
            nc.scalar.dma_start(w2sb[:], w2)
            nc.sync.dma_start(w3sb[:], w3)
            nc.scalar.dma_start(sel[:], sel_d)
            nc.sync.dma_start(fold[:], fold_d)
            nc.scalar.dma_start(bnp[:], bnp_d)
            nc.sync.dma_start(asbsb[:], asb_d)
            for t in (c1[0], c1[1], ybuf[0], ybuf[1], accd):
                nc.gpsimd.memset(t[:], 0.0)
            nc.gpsimd.memset(x2dup[:], 0.0)
            nc.gpsimd.memset(wout[:], 0.0)

            def r3(ap):  # [P, n*72] -> [P, n, 72]
                return ap.rearrange("p (r c) -> p r c", c=PADW)

            def r66(ap):  # [P, n*66] -> [P, n, 66]
                return ap.rearrange("p (r c) -> p r c", c=TW)

            def conv_mms(rhs_of, lhsT_of, nkt, psts, first, last):
                """nkt ktiles x 6 blocks x 9 shifts accumulated into psts.

                rhs is sliced as a strided [K, nr, 66] window (cols 3..69 of
                each 72-col row) — the 8 pad cols per row are never streamed.
                """
                for ti in range(nkt):
                    rhs = r3(rhs_of(ti))
                    for bi, (r0, nr) in enumerate(BLOCKS):
                        for si, (dh, dw) in enumerate(SHIFTS):
                            nc.tensor.matmul(
                                out=psts[bi][:, : nr * TW],
                                lhsT=lhsT_of(ti, si),
                                rhs=rhs[:, r0 + dh : r0 + dh + nr,
                                        TRIM + dw : TRIM + dw + TW],
                                start=(first and ti == 0 and si == 0),
                                stop=(last and ti == nkt - 1 and si == 8),
                            )
                        yield ti, bi

            def evict(psts, dst, m, sc_col, sh_col, base=0):
                """PSUM -> dst with fused BN+ReLU, valid cols only."""
                for bi, (r0, nr) in enumerate(BLOCKS):
                    nc.scalar.activation(
                        out=r3(dst[:m, base : base + PADN])[:, r0 : r0 + nr, 4:68],
                        in_=r66(psts[bi][:m, : nr * TW])[:, :, 1:65],
                        func=mybir.ActivationFunctionType.Relu,
                        scale=bnp[:m, sc_col : sc_col + 1],
                        bias=bnp[:m, sh_col : sh_col + 1],
                    )

            # ---- conv1: 1024 -> 256 ---------------------------------------
            for co in range(2):
                psts = [cps.tile([128, nr * TW], F32, name="cp", tag="cp") for _, nr in BLOCKS]
                for _ in conv_mms(
                    lambda ti: slabs[ti][:],
                    lambda ti, si, _co=co: wst[ti][:, si * 256 + _co * 128 :
                                                  si * 256 + _co * 128 + 128],
                    8, psts, True, True,
                ):
                    pass
                evict(psts, c1[co][:], 128, C1SC + co, C1SH + co)

            # ---- conv2: 256 -> 64, rhs = c1, out -> x2dup upper @DUPB -----
            psts = [cps.tile([64, nr * TW], F32, name="cp", tag="cp") for _, nr in BLOCKS]
            for _ in conv_mms(
                lambda ti: c1[ti][:],
                lambda ti, si: w2sb[:, ti * 576 + si * 64 : ti * 576 + si * 64 + 64],
                2, psts, True, True,
            ):
                pass
            evict(psts, x2dup[:, DUPB : DUPB + PADN], 64, C2SC, C2SH)
            # lower half copy: x2dup[64:128, q] = x2[q] (DMA: cross-partition)
            nc.gpsimd.dma_start(
                x2dup[64:128, 0:PADN], x2dup[0:64, DUPB : DUPB + PADN]
            )

            # refill slabs/wst with ln/w4 (WAR deps force waiting on conv1
            # reads of each tile; Tile inserts them per-tile so refill
            # overlaps the tail of conv1).
            for ti in range(8):
                (nc.scalar if ti % 2 == 0 else nc.sync).dma_start(slabs[ti][:], ln[ti])
                (nc.sync if ti % 2 == 0 else nc.scalar).dma_start(wst[ti][:], w4[ti])

            # ---- conv4 (y branch) interleaved with weighting A_rep/FMA ----
            arep_units = [(g, j) for g in range(NPAIR) for j in range(6)]
            aidx = 0

            def emit_arep(n):
                nonlocal aidx
                for _ in range(n):
                    if aidx >= len(arep_units):
                        return
                    g, j = arep_units[aidx]
                    aidx += 1
                    s = ACC0 + j * WBLK
                    pa = aps.tile([128, WBLK], F32, name="arep", tag="arep")
                    nc.tensor.matmul(
                        out=pa[:], lhsT=sel[:, g * 128 : g * 128 + 128],
                        rhs=asbsb[:, s : s + WBLK],
                        start=True, stop=True,
                    )
                    x = DUPB + _delta(g) + s
                    tt = tmpp.tile([128, WBLK], BF16, name="wt", tag="wt")
                    nc.vector.tensor_tensor(
                        out=tt[:], in0=x2dup[:, x : x + WBLK], in1=pa[:],
                        op=AluOpType.mult,
                    )
                    nc.gpsimd.tensor_tensor(
                        out=accd[:, s : s + WBLK], in0=accd[:, s : s + WBLK],
                        in1=tt[:], op=AluOpType.add,
                    )

            for co in range(2):
                psts = [cps.tile([128, nr * TW], F32, name="cp", tag="cp") for _, nr in BLOCKS]
                for _ti, _bi in conv_mms(
                    lambda ti: slabs[ti][:],
                    lambda ti, si, _co=co: wst[ti][:, si * 256 + _co * 128 :
                                                  si * 256 + _co * 128 + 128],
                    8, psts, True, True,
                ):
                    emit_arep(3)
                evict(psts, ybuf[co][:], 128, C4SC + co, C4SH + co)
            emit_arep(len(arep_units))

            # collapse pair halves: wout = acc_upper + acc_lower (PE fold)
            for o in range(0, PADN, 512):
                n = min(512, PADN - o)
                pc = cps.tile([64, 512], F32, name="cp", tag="cp")
                nc.tensor.matmul(
                    out=pc[:, :n], lhsT=fold[:], rhs=accd[:, o : o + n],
                    start=True, stop=True,
                )
                nc.scalar.activation(
                    out=wout[:, o : o + n], in_=pc[:, :n],
                    func=mybir.ActivationFunctionType.Copy,
                )

            # ---- conv3: 64 -> 256, rhs = wout, out -> c1 ------------------
            for co in range(2):
                psts = [cps.tile([128, nr * TW], F32, name="cp", tag="cp") for _, nr in BLOCKS]
                for _ in conv_mms(
                    lambda ti: wout[:],
                    lambda ti, si, _co=co: w3sb[:, si * 256 + _co * 128 :
                                                si * 256 + _co * 128 + 128],
                    1, psts, True, True,
                ):
                    pass
                evict(psts, c1[co][:], 128, C3SC + co, C3SH + co)

            # ---- d = x3 - y (in place, valid rows incl. pad cols) ---------
            for co in range(2):
                nc.vector.tensor_tensor(
                    out=c1[co][:, ACC0 : ACC0 + ACCN],
                    in0=c1[co][:, ACC0 : ACC0 + ACCN],
                    in1=ybuf[co][:, ACC0 : ACC0 + ACCN],
                    op=AluOpType.subtract,
                )

            # ---- conv5: 256 -> 256 + BN+ReLU + per-row sums ---------------
            nc.sync.dma_start(wst[0][:], w5[0])
            nc.scalar.dma_start(wst[1][:], w5[1])
            for co in range(2):
                psts = [cps.tile([128, nr * TW], F32, name="cp", tag="cp") for _, nr in BLOCKS]
                for _ in conv_mms(
                    lambda ti: c1[ti][:],
                    lambda ti, si, _co=co: wst[ti][:, si * 256 + _co * 128 :
                                                   si * 256 + _co * 128 + 128],
                    2, psts, True, True,
                ):
                    pass
                c5r = c5b.rearrange("p (r c) -> p r c", c=64)
                for bi, (r0, nr) in enumerate(BLOCKS):
                    nc.scalar.activation(
                        out=c5r[:, r0 - 4 : r0 - 4 + nr, :],
                        in_=r66(psts[bi][:, : nr * TW])[:, :, 1:65],
                        func=mybir.ActivationFunctionType.Relu,
                        scale=bnp[:, C5SC + co : C5SC + co + 1],
                        bias=bnp[:, C5SH + co : C5SH + co + 1],
                    )
                nc.vector.tensor_reduce(
                    out=sums[:, co * 40 : co * 40 + 40],
                    in_=c5r[:],
                    axis=AxisListType.X,
                    op=AluOpType.add,
                )
            for co in range(2):
                nc.gpsimd.dma_start(osum[co], sums[:, co * 40 : co * 40 + 40])

    nc.compile()
    return nc


# ------------------------------------------------------------- host side --

def _pad_slab(x_bchw, g0):
    """[1024, 64, 64] -> [8, 128, 48*72] bf16, rows g0-4 .. g0+44."""
    out = np.zeros((1024, PADH, PADW), NPBF16)
    lo, hi = max(0, g0 - HALO), min(H, g0 + ROWS + HALO)
    out[:, lo - (g0 - HALO) : hi - (g0 - HALO), 4:68] = x_bchw[:, lo:hi, :].astype(NPBF16)
    return out.reshape(8, 128, PADN)


def _fold_bn(bn):
    g, b, m, v = [np.asarray(x, np.float32) for x in bn]
    sc = g / np.sqrt(v + BN_EPS)
    return sc, b - m * sc


def _wt(w, nkt):
    """[co, ci, 3, 3] -> [nkt, 128, 9*co] lhsT layout (free = si*Co + co)."""
    co, ci = w.shape[:2]
    return np.ascontiguousarray(
        w.reshape(co, nkt, ci // nkt, 9).transpose(1, 2, 3, 0)
    ).reshape(nkt, ci // nkt, 9 * co).astype(NPBF16)


def prep_core_inputs(inputs, core):
    b, half = core // 2, core % 2
    g0 = half * 24  # local row r = global g0 + r; valid out rows differ
    lk = _pad_slab(np.asarray(inputs["low_key"][b], np.float32), g0)
    ln = _pad_slab(np.asarray(inputs["low_nonkey"][b], np.float32), g0)

    att = np.asarray(inputs["local_atten"][b], np.float32)  # [64, 64, 81]
    asb = np.zeros((2 * NPAIR, PADN), NPBF16)
    a_loc = np.zeros((81, PADH, PADW), np.float32)
    a_loc[:, 4 : 4 + ROWS, 4:68] = att[g0 : g0 + ROWS].transpose(2, 0, 1)
    a_loc = a_loc.reshape(81, PADN)
    for g in range(NPAIR):
        asb[2 * g] = a_loc[g].astype(NPBF16)
        if g + NPAIR < 81:
            asb[2 * g + 1] = a_loc[g + NPAIR].astype(NPBF16)

    w2t = _wt(np.asarray(inputs["w2"]), 2)  # [2, 128, 576]
    w2p = np.concatenate([w2t[0], w2t[1]], axis=1)  # [128, 1152]

    sel = np.zeros((2 * NPAIR, NPAIR * 128), NPBF16)
    for g in range(NPAIR):
        sel[2 * g, g * 128 : g * 128 + 64] = 1.0
        sel[2 * g + 1, g * 128 + 64 : g * 128 + 128] = 1.0
    fold = np.zeros((128, 64), NPBF16)
    fold[np.arange(64), np.arange(64)] = 1.0
    fold[64 + np.arange(64), np.arange(64)] = 1.0

    bnp = np.zeros((128, 18), np.float32)
    for i, (name, cols) in enumerate(
        [("bn1", (0, 2)), ("bn2", (4, 5)), ("bn3", (6, 8)),
         ("bn4", (10, 12)), ("bn5", (14, 16))]
    ):
        sc, sh = _fold_bn(np.asarray(inputs[name]))
        nco = sc.shape[0]
        if nco == 256:
            bnp[:, cols[0]] = sc[:128]
            bnp[:, cols[0] + 1] = sc[128:]
            bnp[:, cols[1]] = sh[:128]
            bnp[:, cols[1] + 1] = sh[128:]
        else:
            bnp[:64, cols[0]] = sc
            bnp[:64, cols[1]] = sh

    return {
        "lk": lk, "ln": ln, "asb": asb,
        "w1": _wt(np.asarray(inputs["w1"]), 8),
        "w2": w2p,
        "w3": _wt(np.asarray(inputs["w3"]), 1)[0],  # [64, 2304]
        "w4": _wt(np.asarray(inputs["w4"]), 8),
        "w5": _wt(np.asarray(inputs["w5"]), 2),
        "sel": sel, "fold": fold, "bnp": bnp,
    }


def postprocess(osums, inputs):
    """osums: list of 8 arrays [2, 128, 40] -> final [4, 1]."""
    mean = np.zeros((4, 256), np.float32)
    for core in range(8):
        b, half = core // 2, core % 2
        s = osums[core].reshape(256, 40)
        rows = slice(0, 32) if half == 0 else slice(8, 40)
        mean[b] += s[:, rows].sum(axis=1)
    mean /= float(H * W)
    fw1 = np.asarray(inputs["fw1"], np.float32)
    fb1 = np.asarray(inputs["fb1"], np.float32)
    fw2 = np.asarray(inputs["fw2"], np.float32)
    fb2 = np.asarray(inputs["fb2"], np.float32)
    out = mean @ fw1.T + fb1
    out = out @ fw2.T + fb2
    return out.astype(np.float32)


_prog_cache = {}
LAST = {}


def kernel(**inputs) -> np.ndarray:
    import os, time
    if "nc" not in _prog_cache:
        _prog_cache["nc"] = _build_program()
    nc = _prog_cache["nc"]
    in_maps = [prep_core_inputs(inputs, core) for core in range(8)]
    t0 = time.time()
    res = run_bass_kernel_spmd(
        nc, in_maps, list(range(8)), trace=bool(os.environ.get("KS_TRACE"))
    )
    LAST["spmd_s"] = time.time() - t0
    LAST["res"] = res
    return postprocess([r["osum"] for r in res.results], inputs)
